# revision 1
# baseline (speedup 1.0000x reference)
"""Tensor-parallel attention block (QKV + RoPE + causal attention + out-proj)
for 8 Trainium2 NeuronCores.

Sharding: heads (16) split across 8 cores, 2 heads/core. wq/wk/wv column-
sharded, wo row-sharded; each core computes a full-shape partial output and
the host sums the 8 partials.

Layout trick: everything on the PE array is a natural `lhsT.T @ rhs`:
  - host pre-transposes x -> xT [D, B*S] so projections need no transposes
  - q,k produced in [head_dim, seq] layout; scores computed TRANSPOSED
    ([k_seq, q_seq]) so softmax needs no on-chip transposes at all
  - denominator = ones[128,128] matmul over probsT chunks -> broadcast rows
  - RoPE pair-halves are deinterleaved by permuting wq/wk rows on host;
    the half-swap needed by rotation is done with an SBUF->SBUF DMA
  - mask handled exactly as multiplicative exp(mask) tiles; all-zero tiles
    skip compute, all-one tiles skip the multiply (derived from the real
    mask values at build time, not assumed causal)
"""

import math
import os
import sys

import numpy as np
import ml_dtypes

sys.path.insert(0, "/opt/trn_rl_repo")

import concourse.bass as bass
import concourse.mybir as mybir
from concourse.tile import TileContext
from concourse.bass_utils import run_bass_kernel_spmd
from concourse.masks import make_identity

B, S, D, H = 2, 2048, 2048, 16
HD = D // H            # 128 head dim
NCORES = 8
HC = H // NCORES       # 2 heads per core
DHC = HC * HD          # 256
BS = B * S             # 4096
NDIN = D // 128        # 16 contraction chunks
W = 512                # attention q-window / matmul free size
NQW = S // W           # 4 q windows per batch
NKC = S // 128         # 16 k chunks per batch
SG = 1024              # qkv seq-group width
NSG = BS // SG         # 4
RSQRT_HD = 1.0 / math.sqrt(HD)

BF16 = mybir.dt.bfloat16
F32 = mybir.dt.float32
NPBF16 = ml_dtypes.bfloat16

SKIP, FREE, MASKED = 0, 1, 2

# stash of the last BassKernelResults for the test harness (exec_time_ns etc)
LAST_RUN = [None]
_PROGRAM_CACHE = {}


def _split_multi_waits(nc):
    """Walrus codegen allows only 1 embedded sync-wait per instruction (2 for
    EventSemaphore). Tile's sem-assignment can emit more; hoist the excess into
    standalone InstEventSemaphore waits on the same engine, just before."""
    n = 0
    for blk in nc.m.functions[0].blocks:
        out = []
        for inst in blk.instructions:
            si = getattr(inst, "sync_info", None)
            cap = 2 if isinstance(inst, mybir.InstEventSemaphore) else 1
            if si is not None and si.on_wait and len(si.on_wait) > cap:
                waits = list(si.on_wait)
                for w in waits[:-cap]:
                    n += 1
                    ev = mybir.InstEventSemaphore(
                        name=f"{inst.name}_xw{n}",
                        ins=[], outs=[],
                        sync_info=mybir.SyncInfo(on_wait=[w], on_update=[]))
                    ev.engine = inst.engine
                    out.append(ev)
                si.on_wait = waits[-cap:]
            out.append(inst)
        blk.instructions = out


def _build(cls_key):
    """Build the per-core Bass program. cls_key: tuple[NQW][NKC] of SKIP/FREE/MASKED."""
    cls = [list(row) for row in cls_key]
    nc = bass.Bass()

    xT = nc.declare_dram_parameter("xT", [D, BS], BF16, isOutput=False)
    wqT = nc.declare_dram_parameter("wqT", [D, DHC], BF16, isOutput=False)
    wkT = nc.declare_dram_parameter("wkT", [D, DHC], BF16, isOutput=False)
    wvT = nc.declare_dram_parameter("wvT", [D, DHC], BF16, isOutput=False)
    woT = nc.declare_dram_parameter("woT", [DHC, D], BF16, isOutput=False)
    trig = nc.declare_dram_parameter("trig", [128, 2 * S], F32, isOutput=False)
    emaskT = nc.declare_dram_parameter("emaskT", [S, S], BF16, isOutput=False)
    out_d = nc.declare_dram_parameter("out", [BS, D], F32, isOutput=True)

    with TileContext(nc) as tc:
        with (
            tc.tile_pool(name="consts", bufs=1) as consts,
            tc.tile_pool(name="xt", bufs=5) as xtp,
            tc.tile_pool(name="rsw", bufs=2) as rswp,
            tc.tile_pool(name="rm", bufs=2) as rmp,
            tc.tile_pool(name="vtmp", bufs=2) as vtp,
            tc.tile_pool(name="probs", bufs=5) as prp,
            tc.tile_pool(name="emask", bufs=6) as emp,
            tc.tile_pool(name="small", bufs=2) as smp,
            tc.tile_pool(name="outsb", bufs=3) as outp,
            tc.tile_pool(name="psA", bufs=2, space="PSUM") as psA,
            tc.tile_pool(name="psB", bufs=2, space="PSUM") as psB,
            tc.tile_pool(name="psC", bufs=4, space="PSUM") as psC,
        ):
            # persistent tiles
            q_sb = [consts.tile([128, BS], BF16, tag=f"q{h}", name=f"q{h}") for h in range(HC)]
            k_sb = [consts.tile([128, BS], BF16, tag=f"k{h}", name=f"k{h}") for h in range(HC)]
            a_sb = [consts.tile([128, BS], BF16, tag=f"a{h}", name=f"a{h}") for h in range(HC)]
            vT_sb = consts.tile([128, B * NKC * DHC], BF16, tag="vT", name="vT")
            ident = consts.tile([128, 128], BF16, tag="ident", name="ident")
            make_identity(nc, ident)
            ones = consts.tile([128, 128], BF16, tag="ones", name="ones")
            nc.vector.memset(ones, 1.0)

            # warm the PE clock (HAM releases the 1.2GHz throttle after ~3.4us
            # of sustained activity) while the first DMAs are in flight
            wu = psC.tile([128, 128], F32, tag="ad", name="warmup")
            for i in range(120):
                nc.tensor.matmul(wu, lhsT=ones, rhs=ones, start=True, stop=True)

            # qkv weights resident: [128, NDIN, DHC], col block di = wT[di*128:+128, :]
            w_all = []
            for wi_, wparam in enumerate([wqT, wkT, wvT]):
                wt_ = consts.tile([128, NDIN, DHC], BF16, tag=f"wall{wi_}", name=f"wall{wi_}")
                nc.gpsimd.dma_start(
                    out=wt_, in_=wparam.rearrange("(n p) m -> p n m", p=128))
                w_all.append(wt_)
            trig_sb = consts.tile([128, 2 * S], F32, tag="trig", name="trig")
            woT_sb = [consts.tile([128, D], BF16, tag=f"wo{h}", name=f"wo{h}")
                      for h in range(HC)]

            # ---- QKV projections ----
            for sg in range(NSG):
                xts = []
                for dj in range(4):
                    tb = xtp.tile([128, 4, SG], BF16, tag="xt", name=f"xt{sg}_{dj}")
                    nc.gpsimd.dma_start(
                        out=tb,
                        in_=xT[dj * 512:(dj + 1) * 512,
                               sg * SG:(sg + 1) * SG].rearrange("(n p) m -> p n m", p=128))
                    for k4 in range(4):
                        xts.append(tb[:, k4, :])
                if sg == 0:
                    # needed only from rope onward; keep it out of the startup
                    # DMA critical path
                    nc.gpsimd.dma_start(out=trig_sb, in_=trig[:, :])
                if sg == 1:
                    for h in range(HC):
                        nc.gpsimd.dma_start(out=woT_sb[h], in_=woT[h * 128:(h + 1) * 128, :])
                for ti in range(3):
                    for dh in range(2):
                        ps = [psA.tile([128, W], F32, tag="psA", name=f"psA{sg}_{ti}_{dh}_{wi}")
                              for wi in range(2)]
                        for di in range(NDIN):
                            for wi in range(2):
                                nc.tensor.matmul(
                                    ps[wi], lhsT=w_all[ti][:, di, dh * 128:(dh + 1) * 128],
                                    rhs=xts[di][:, wi * W:(wi + 1) * W],
                                    start=(di == 0), stop=(di == NDIN - 1))
                        for wi in range(2):
                            c0 = sg * SG + wi * W
                            if ti < 2:
                                dst = (q_sb if ti == 0 else k_sb)[dh]
                                with tc.high_priority():
                                    nc.scalar.copy(dst[:, c0:c0 + W], ps[wi])
                            else:
                                vt = vtp.tile([128, W], BF16, tag="vtmp", name=f"vt{sg}_{dh}_{wi}")
                                with tc.high_priority():
                                    nc.scalar.copy(vt, ps[wi])
                                for j in range(W // 128):
                                    pt = psC.tile([128, 128], BF16, tag="ad",
                                                  name=f"pvt{sg}_{dh}_{wi}_{j}")
                                    nc.tensor.transpose(pt, vt[:, j * 128:(j + 1) * 128], ident)
                                    g = (c0 + j * 128) // 128
                                    o0 = g * DHC + dh * 128
                                    nc.scalar.copy(vT_sb[:, o0:o0 + 128], pt)
                # rope for batch b once its two seq-groups are projected
                if sg % 2 == 1:
                    b = sg // 2
                    for tens in (q_sb, k_sb):
                        for h in range(HC):
                            src = tens[h]
                            cc = b * S
                            sw = rswp.tile([128, S], BF16, tag="rsw", name=f"sw{sg}_{h}")
                            nc.gpsimd.dma_start(out=sw[0:64, :], in_=src[64:128, cc:cc + S])
                            nc.gpsimd.dma_start(out=sw[64:128, :], in_=src[0:64, cc:cc + S])
                            mcc = rmp.tile([128, S], BF16, tag="mcc", name=f"mcc{sg}_{h}")
                            mss = rmp.tile([128, S], BF16, tag="mss", name=f"mss{sg}_{h}")
                            nc.vector.tensor_mul(mcc, src[:, cc:cc + S], trig_sb[:, 0:S])
                            nc.vector.tensor_mul(mss, sw, trig_sb[:, S:2 * S])
                            nc.vector.tensor_add(src[:, cc:cc + S], mcc, mss)

            # ---- attention (scores transposed: [k_seq, q_seq]) ----
            # b outer: batch-0 attention (ACT-heavy) overlaps batch-1 QKV
            # (PE-heavy); batch-1 attention overlaps batch-0 out-projection.
            for b in range(B):
                for qw in range(NQW):
                    active = [c for c in range(NKC) if cls[qw][c][0] != SKIP]
                    em_tiles = {}
                    for c in active:
                        if cls[qw][c][0] == MASKED:
                            em = emp.tile([128, W], BF16, tag="em", name=f"em{b}_{qw}_{c}")
                            nc.gpsimd.dma_start(
                                out=em,
                                in_=emaskT[c * 128:(c + 1) * 128, qw * W:(qw + 1) * W])
                            em_tiles[c] = em
                    for h in range(HC):
                        if not active:
                            continue
                        qc = b * S + qw * W
                        att = psC.tile([128, W], F32, tag="ad", name=f"att{b}_{h}_{qw}")
                        dsm = psC.tile([128, W], F32, tag="ad", name=f"dsm{b}_{h}_{qw}")
                        for ci, c in enumerate(active):
                            kind, off = cls[qw][c]
                            sp = psB.tile([128, W], F32, tag="psB", name=f"sc{b}_{h}_{qw}_{c}")
                            kc = b * S + c * 128
                            nc.tensor.matmul(sp, lhsT=k_sb[h][:, kc:kc + 128],
                                             rhs=q_sb[h][:, qc:qc + W],
                                             start=True, stop=True)
                            pb = prp.tile([128, W], BF16, tag="probs", name=f"pb{b}_{h}_{qw}_{c}")
                            nc.scalar.activation(pb, sp,
                                                 mybir.ActivationFunctionType.Exp,
                                                 scale=RSQRT_HD)
                            if kind == MASKED:
                                nc.vector.tensor_mul(pb, pb, em_tiles[c])
                            g = b * NKC + c
                            o0 = g * DHC + h * 128
                            nc.tensor.matmul(att, lhsT=vT_sb[:, o0:o0 + 128], rhs=pb,
                                             start=(ci == 0), stop=(ci == len(active) - 1))
                            nc.tensor.matmul(dsm, lhsT=ones, rhs=pb,
                                             start=(ci == 0), stop=(ci == len(active) - 1))
                        rc = smp.tile([128, W], F32, tag="recip", name=f"rc{b}_{h}_{qw}")
                        with tc.high_priority():
                            nc.vector.reciprocal(rc, dsm)
                            nc.vector.tensor_mul(a_sb[h][:, qc:qc + W], att, rc)

            # ---- output projection (partial over this core's heads) ----
            for st in range(BS // 128):
                for dgg in range(2):
                    ops = [psA.tile([128, W], F32, tag="psA", name=f"o{st}_{dgg}_{d2}")
                           for d2 in range(2)]
                    for h in range(HC):
                        for d2 in range(2):
                            dg = dgg * 2 + d2
                            nc.tensor.matmul(
                                ops[d2], lhsT=a_sb[h][:, st * 128:(st + 1) * 128],
                                rhs=woT_sb[h][:, dg * W:(dg + 1) * W],
                                start=(h == 0), stop=(h == HC - 1))
                    for d2 in range(2):
                        dg = dgg * 2 + d2
                        ob = outp.tile([128, W], F32, tag="ob", name=f"ob{st}_{dg}")
                        with tc.high_priority():
                            if (st + dg) % 2 == 0:
                                nc.scalar.copy(ob, ops[d2])
                            else:
                                nc.vector.tensor_copy(ob, ops[d2])
                        nc.gpsimd.dma_start(
                            out=out_d[st * 128:(st + 1) * 128, dg * W:(dg + 1) * W], in_=ob)
    _split_multi_waits(nc)
    return nc


def _prepare(x, freqs_cos, freqs_sin, mask, wq, wk, wv, wo):
    x = np.asarray(x, dtype=np.float32)
    wq = np.asarray(wq, dtype=np.float32)
    wk = np.asarray(wk, dtype=np.float32)
    wv = np.asarray(wv, dtype=np.float32)
    wo = np.asarray(wo, dtype=np.float32)
    fc = np.asarray(freqs_cos, dtype=np.float32)
    fs = np.asarray(freqs_sin, dtype=np.float32)
    mask = np.asarray(mask, dtype=np.float32)

    xT = np.ascontiguousarray(x.reshape(BS, D).T).astype(NPBF16)

    cosT = fc.T                      # [64, S]
    sinT = fs.T
    cos_dup = np.vstack([cosT, cosT])
    sin_sgn = np.vstack([-sinT, sinT])
    trig = np.ascontiguousarray(np.hstack([cos_dup, sin_sgn])).astype(np.float32)

    em = np.exp(mask).T              # [k, q]; exp(-inf)=0, exp(0)=1
    emaskT = np.ascontiguousarray(em).astype(NPBF16)
    cls = []
    for qw in range(NQW):
        row = []
        for c in range(NKC):
            t = emaskT[c * 128:(c + 1) * 128, qw * W:(qw + 1) * W]
            if not t.any():
                row.append((SKIP, 0))
            elif (t == NPBF16(1.0)).all():
                row.append((FREE, 0))
            else:
                colnz = (np.asarray(t, dtype=np.float32) != 0).any(axis=0)
                off = int(np.argmax(colnz))  # first column with any valid entry
                row.append((MASKED, off))
        cls.append(tuple(row))
    cls_key = tuple(cls)

    # deinterleave perm: even dims then odd dims, per head
    ridx = np.concatenate([np.arange(0, HD, 2), np.arange(1, HD, 2)])
    in_maps = []
    for core in range(NCORES):
        heads = [core * HC + h for h in range(HC)]
        qk_rows = np.concatenate([g * HD + ridx for g in heads])
        v_rows = np.concatenate([np.arange(g * HD, (g + 1) * HD) for g in heads])
        m = {
            "xT": xT,
            "wqT": np.ascontiguousarray(wq[qk_rows].T).astype(NPBF16),
            "wkT": np.ascontiguousarray(wk[qk_rows].T).astype(NPBF16),
            "wvT": np.ascontiguousarray(wv[v_rows].T).astype(NPBF16),
            "woT": np.ascontiguousarray(wo[:, v_rows].T).astype(NPBF16),
            "trig": trig,
            "emaskT": emaskT,
        }
        in_maps.append(m)
    return in_maps, cls_key


def kernel(x, start_pos, freqs_cos, freqs_sin, mask, wq, wk, wv, wo):
    in_maps, cls_key = _prepare(x, freqs_cos, freqs_sin, mask, wq, wk, wv, wo)
    nc = _PROGRAM_CACHE.get(cls_key)
    if nc is None:
        nc = _build(cls_key)
        _PROGRAM_CACHE[cls_key] = nc
    res = run_bass_kernel_spmd(
        nc, in_maps, list(range(NCORES)),
        trace=bool(os.environ.get("KERNEL_TRACE")),
        tmpdir=os.environ.get("KERNEL_TRACE_DIR") or None)
    LAST_RUN[0] = res
    out = np.zeros([BS, D], np.float32)
    for r in res.results:
        out += np.asarray(r["out"], dtype=np.float32)
    return out.reshape(B, S, D)



# revision 11
# speedup vs baseline: 1.2415x; 1.2415x over previous
"""Tensor-parallel attention block (QKV + RoPE + causal attention + out-proj)
for 8 Trainium2 NeuronCores.

Sharding: heads (16) split across 8 cores, 2 heads/core. wq/wk/wv column-
sharded, wo row-sharded; each core computes a full-shape partial output (bf16)
and the host sums the 8 partials in f32.

Differences vs the v1 kernel (measured on HW: PE runs at 2.0 GHz warm /
1.0 GHz cold on this part, and the kernel is PE-streaming-cycle bound):
  - denominator of softmax moved off the PE: DVE accumulates exp chunks
    (bf16) and two small ones-matmuls per (b,qw) group broadcast the
    column sums; reciprocal via the fast DVE approx (the exact DVE
    reciprocal is 8 cyc/elem).
  - dead causal regions skipped at column granularity: each (qw, kc) tile
    only computes scores/exp/AV over q >= off (off from the real mask).
  - V projection emitted directly in [seq, hd] layout (stationary = x
    chunk), killing the PE transposes and their PSUM->SBUF copies.
  - out-projection interleaved per (b, qw) window right after attention of
    that window, so output DMA overlaps compute and the PE never idles
    long enough to re-throttle (HAM) at the end.
  - both heads packed side by side in [128, 2, 512] tiles: one ACT exp,
    one DVE recip/mul per group instead of per head.
  - output written bf16; out DMAs are [128, 2048] on the SP (sync) hwdge
    queue; input weight loads also on SP to parallelize with x loads.
"""

import math
import os
import sys

import numpy as np
import ml_dtypes

sys.path.insert(0, "/opt/trn_rl_repo")

import concourse.bass as bass
import concourse.mybir as mybir
from concourse.tile import TileContext
from concourse.bass_utils import run_bass_kernel_spmd

B, S, D, H = 2, 2048, 2048, 16
HD = D // H            # 128 head dim
NCORES = 8
HC = H // NCORES       # 2 heads per core
DHC = HC * HD          # 256
BS = B * S             # 4096
NDIN = D // 128        # 16 contraction chunks
W = 512                # attention q-window
NQW = S // W           # 4 q windows per batch
NKC = S // 128         # 16 k chunks per batch
SG = 1024              # qkv seq-group width
NSG = BS // SG         # 4
RSQRT_HD = 1.0 / math.sqrt(HD)

BF16 = mybir.dt.bfloat16
F32 = mybir.dt.float32
NPBF16 = ml_dtypes.bfloat16

SKIP, FREE, MASKED = 0, 1, 2

# stash of the last BassKernelResults for the test harness (exec_time_ns etc)
LAST_RUN = [None]
_PROGRAM_CACHE = {}


def _split_multi_waits(nc):
    """Walrus codegen allows only 1 embedded sync-wait per instruction (2 for
    EventSemaphore). Tile's sem-assignment can emit more; hoist the excess into
    standalone InstEventSemaphore waits on the same engine, just before."""
    n = 0
    for blk in nc.m.functions[0].blocks:
        out = []
        for inst in blk.instructions:
            si = getattr(inst, "sync_info", None)
            cap = 2 if isinstance(inst, mybir.InstEventSemaphore) else 1
            if si is not None and si.on_wait and len(si.on_wait) > cap:
                waits = list(si.on_wait)
                for w in waits[:-cap]:
                    n += 1
                    ev = mybir.InstEventSemaphore(
                        name=f"{inst.name}_xw{n}",
                        ins=[], outs=[],
                        sync_info=mybir.SyncInfo(on_wait=[w], on_update=[]))
                    ev.engine = inst.engine
                    out.append(ev)
                si.on_wait = waits[-cap:]
            out.append(inst)
        blk.instructions = out


def _build(cls_key, n_em):
    """Build the per-core Bass program.

    cls_key: tuple[NQW][NKC] of (kind, off, em_id); n_em: number of unique
    deduped [128, 2, 512] em blocks shipped in the `emu` dram tensor."""
    cls = [list(row) for row in cls_key]
    nc = bass.Bass()

    xT = nc.declare_dram_parameter("xT", [D, BS], BF16, isOutput=False)
    wqT = nc.declare_dram_parameter("wqT", [D, DHC], BF16, isOutput=False)
    wkT = nc.declare_dram_parameter("wkT", [D, DHC], BF16, isOutput=False)
    wvT = nc.declare_dram_parameter("wvT", [D, DHC], BF16, isOutput=False)
    woT = nc.declare_dram_parameter("woT", [DHC, D], BF16, isOutput=False)
    trig = nc.declare_dram_parameter("trig", [128, 2 * S], BF16, isOutput=False)
    emu = nc.declare_dram_parameter("emu", [128, max(n_em, 1), 2, W], BF16,
                                    isOutput=False)
    out_d = nc.declare_dram_parameter("out", [BS, D], BF16, isOutput=True)

    with TileContext(nc) as tc:
        with (
            tc.tile_pool(name="consts", bufs=1) as consts,
            tc.tile_pool(name="xt", bufs=5) as xtp,
            tc.tile_pool(name="rsw", bufs=2) as rswp,
            tc.tile_pool(name="rtmp", bufs=2) as rtp,
            tc.tile_pool(name="pb", bufs=4) as pbp,
            tc.tile_pool(name="acc", bufs=2) as accp,
            tc.tile_pool(name="apair", bufs=3) as app,
            tc.tile_pool(name="rc", bufs=2) as rcp,
            tc.tile_pool(name="ob", bufs=2) as obp,
            tc.tile_pool(name="psBig", bufs=2, space="PSUM") as psBig,
            tc.tile_pool(name="psAcc", bufs=2, space="PSUM") as psAcc,
        ):
            # ---- persistent tiles ----
            q_sb = [consts.tile([128, BS], BF16, tag=f"q{h}", name=f"q{h}") for h in range(HC)]
            k_sb = [consts.tile([128, BS], BF16, tag=f"k{h}", name=f"k{h}") for h in range(HC)]
            vT_sb = consts.tile([128, B * NKC * DHC], BF16, tag="vT", name="vT")
            ones = consts.tile([128, 128], BF16, tag="ones", name="ones")
            nc.vector.memset(ones, 1.0)

            # warm the PE clock (HAM releases the throttle after ~3.4us of
            # sustained activity) while the first DMAs are in flight
            wu = psAcc.tile([128, 128], F32, tag="ps2", name="warmup")
            for i in range(40):
                nc.tensor.matmul(wu, lhsT=ones, rhs=ones, start=True, stop=True)

            # weights resident: [128, NDIN, DHC], halves loaded separately so
            # the first projection matmuls can start sooner
            w_all = []
            for wi_, wparam in enumerate([wqT, wkT, wvT]):
                wt_ = consts.tile([128, NDIN, DHC], BF16, tag=f"wall{wi_}", name=f"wall{wi_}")
                for hf in range(2):
                    r0 = hf * (D // 2)
                    nc.sync.dma_start(
                        out=wt_[:, hf * (NDIN // 2):(hf + 1) * (NDIN // 2), :],
                        in_=wparam[r0:r0 + D // 2, :].rearrange("(n p) m -> p n m", p=128))
                w_all.append(wt_)
            trig_sb = consts.tile([128, 2 * S], BF16, tag="trig", name="trig")
            nc.sync.dma_start(out=trig_sb, in_=trig[:, :])
            woT_sb = [consts.tile([128, D], BF16, tag=f"wo{h}", name=f"wo{h}")
                      for h in range(HC)]
            for h in range(HC):
                nc.sync.dma_start(out=woT_sb[h], in_=woT[h * 128:(h + 1) * 128, :])
            em_sb = []
            for e in range(n_em):
                emt = consts.tile([128, 2, W], BF16, tag=f"em{e}", name=f"em{e}")
                nc.sync.dma_start(out=emt, in_=emu[:, e, :, :])
                em_sb.append(emt)

            def rope_unit(b, tens, h):
                src = tens[h]
                cc = b * S
                sw = rswp.tile([128, S], BF16, tag="rsw", name=f"sw{b}_{h}")
                nc.gpsimd.dma_start(out=sw[0:64, :], in_=src[64:128, cc:cc + S])
                nc.gpsimd.dma_start(out=sw[64:128, :], in_=src[0:64, cc:cc + S])
                mcc = rtp.tile([128, S], BF16, tag="mcc", name=f"mcc{b}_{h}")
                nc.vector.tensor_mul(mcc, src[:, cc:cc + S], trig_sb[:, 0:S])
                nc.vector.tensor_mul(sw, sw, trig_sb[:, S:2 * S])
                nc.vector.tensor_add(src[:, cc:cc + S], mcc, sw)

            # ---- QKV projections (per batch) ----
            for b in range(B):
                for sg in range(2 * b, 2 * b + 2):
                    xts = []
                    for dj in range(4):
                        tb = xtp.tile([128, 4, SG], BF16, tag="xt", name=f"xt{sg}_{dj}")
                        nc.gpsimd.dma_start(
                            out=tb,
                            in_=xT[dj * 512:(dj + 1) * 512,
                                   sg * SG:(sg + 1) * SG].rearrange("(n p) m -> p n m", p=128))
                        for k4 in range(4):
                            xts.append(tb[:, k4, :])
                    # q, k: stationary = weight tile, moving = x
                    for ti in range(2):
                        for dh in range(HC):
                            ps = psBig.tile([128, 2, W], F32, tag="psBig",
                                            name=f"ps{sg}_{ti}_{dh}")
                            for di in range(NDIN):
                                for wi in range(2):
                                    nc.tensor.matmul(
                                        ps[:, wi, :],
                                        lhsT=w_all[ti][:, di, dh * 128:(dh + 1) * 128],
                                        rhs=xts[di][:, wi * W:(wi + 1) * W],
                                        start=(di == 0), stop=(di == NDIN - 1))
                            dst = (q_sb if ti == 0 else k_sb)[dh]
                            with tc.high_priority():
                                nc.scalar.copy(
                                    dst[:, sg * SG:(sg + 1) * SG].rearrange(
                                        "p (n m) -> p n m", n=2),
                                    ps)
                    # v: stationary = x chunk, moving = wv cols -> [seq, hd]
                    for sc in range(SG // 128):
                        vps = psAcc.tile([128, DHC], F32, tag="ps2",
                                         name=f"vps{sg}_{sc}")
                        for di in range(NDIN):
                            nc.tensor.matmul(
                                vps, lhsT=xts[di][:, sc * 128:(sc + 1) * 128],
                                rhs=w_all[2][:, di, :],
                                start=(di == 0), stop=(di == NDIN - 1))
                        g = sg * (SG // 128) + sc
                        with tc.high_priority():
                            nc.scalar.copy(vT_sb[:, g * DHC:(g + 1) * DHC], vps)
                # rope for this batch; b=1's units are emitted interleaved
                # into attention-b0 groups (DVE would otherwise stall the
                # first attention denominators behind 24 rope ops)
                if b == 0:
                    for tens in (q_sb, k_sb):
                        for h in range(HC):
                            rope_unit(0, tens, h)

            # ---- attention + out-projection, interleaved per (b, qw) ----
            rope_b1 = [(tens, h) for tens in (q_sb, k_sb) for h in range(HC)]
            for b in range(B):
                for qw in range(NQW):
                    if b == 0:
                        ru = rope_b1[qw:qw + 1]
                        for tens, h in ru:
                            rope_unit(1, tens, h)
                    active = [c for c in range(NKC) if cls[qw][c][0] != SKIP]
                    if not active:
                        continue
                    qc = b * S + qw * W
                    att = psAcc.tile([128, 2, W], F32, tag="ps2",
                                     name=f"att{b}_{qw}")
                    acc = accp.tile([128, 2, W], BF16, tag="acc",
                                    name=f"acc{b}_{qw}")
                    nact = len(active)
                    # denominator accumulation engine alternates per group:
                    # gpsimd is otherwise idle and the chain only gates the
                    # group-end broadcast matmul
                    acc_eng = nc.vector if (b * NQW + qw) % 2 == 0 else nc.gpsimd
                    for ci, c in enumerate(active):
                        kind, off, em_id = cls[qw][c]
                        if ci == 0:
                            # first chunk computes full width so every PSUM
                            # column gets its start=True write and acc its
                            # full init (exact for arbitrary masks: em=0
                            # zeroes invalid entries)
                            off = 0
                        sp = psBig.tile([128, 2, W], F32, tag="psBig",
                                        name=f"sc{b}_{qw}_{c}")
                        kc = b * S + c * 128
                        for h in range(HC):
                            nc.tensor.matmul(sp[:, h, off:W],
                                             lhsT=k_sb[h][:, kc:kc + 128],
                                             rhs=q_sb[h][:, qc + off:qc + W],
                                             start=True, stop=True)
                        pb = pbp.tile([128, 2, W], BF16, tag="pb",
                                      name=f"pb{b}_{qw}_{c}")
                        with tc.high_priority():
                            nc.scalar.activation(pb[:, :, off:W], sp[:, :, off:W],
                                                 mybir.ActivationFunctionType.Exp,
                                                 scale=RSQRT_HD)
                        if kind == MASKED:
                            nc.vector.tensor_mul(pb[:, :, off:W], pb[:, :, off:W],
                                                 em_sb[em_id][:, :, off:W])
                        # denominator partial sums (off the PE)
                        if ci == 0:
                            acc_eng.tensor_copy(acc, pb)
                        else:
                            acc_eng.tensor_add(acc[:, :, off:W],
                                               acc[:, :, off:W],
                                               pb[:, :, off:W])
                        g = b * NKC + c
                        for h in range(HC):
                            nc.tensor.matmul(
                                att[:, h, off:W],
                                lhsT=vT_sb[:, g * DHC + h * 128:g * DHC + (h + 1) * 128],
                                rhs=pb[:, h, off:W],
                                start=(ci == 0), stop=(ci == nact - 1))
                    # broadcast column sums, reciprocal, normalize
                    dsm = psBig.tile([128, 2, W], F32, tag="psBig",
                                     name=f"dsm{b}_{qw}")
                    for h in range(HC):
                        nc.tensor.matmul(dsm[:, h, :], lhsT=ones, rhs=acc[:, h, :],
                                         start=True, stop=True)
                    # 1/dsm = exp(-ln(dsm)): two ACT LUT passes, ~5x cheaper
                    # than the exact DVE reciprocal (8 cyc/elem) and well
                    # within the error budget for softmax denominators
                    lnd = rcp.tile([128, 2, W], F32, tag="rc", name=f"ln{b}_{qw}")
                    rc = rcp.tile([128, 2, W], F32, tag="rc", name=f"rc{b}_{qw}")
                    a_pair = app.tile([128, 2, W], BF16, tag="apair",
                                      name=f"ap{b}_{qw}")
                    with tc.high_priority():
                        nc.scalar.activation(lnd, dsm,
                                             mybir.ActivationFunctionType.Ln)
                        nc.scalar.activation(rc, lnd,
                                             mybir.ActivationFunctionType.Exp,
                                             scale=-1.0)
                        nc.vector.tensor_mul(a_pair, att, rc)
                    # out-projection for this window's 512 rows
                    for st in range(W // 128):
                        ops = [psAcc.tile([128, 2, W], F32, tag="ps2",
                                          name=f"o{b}_{qw}_{st}_{t}")
                               for t in range(2)]
                        for h in range(HC):
                            for t in range(2):
                                for j in range(2):
                                    dg = t * 2 + j
                                    nc.tensor.matmul(
                                        ops[t][:, j, :],
                                        lhsT=a_pair[:, h, st * 128:(st + 1) * 128],
                                        rhs=woT_sb[h][:, dg * W:(dg + 1) * W],
                                        start=(h == 0), stop=(h == HC - 1))
                        ob = obp.tile([128, 4, W], BF16, tag="ob",
                                      name=f"ob{b}_{qw}_{st}")
                        with tc.high_priority():
                            nc.scalar.copy(ob[:, 0:2, :], ops[0])
                            nc.vector.tensor_copy(ob[:, 2:4, :], ops[1])
                        r0 = b * S + qw * W + st * 128
                        nc.sync.dma_start(
                            out=out_d[r0:r0 + 128, :],
                            in_=ob.rearrange("p a b -> p (a b)"))
    _split_multi_waits(nc)
    return nc


def _prepare(x, freqs_cos, freqs_sin, mask, wq, wk, wv, wo):
    x = np.asarray(x, dtype=np.float32)
    wq = np.asarray(wq, dtype=np.float32)
    wk = np.asarray(wk, dtype=np.float32)
    wv = np.asarray(wv, dtype=np.float32)
    wo = np.asarray(wo, dtype=np.float32)
    fc = np.asarray(freqs_cos, dtype=np.float32)
    fs = np.asarray(freqs_sin, dtype=np.float32)
    mask = np.asarray(mask, dtype=np.float32)

    xT = np.ascontiguousarray(x.reshape(BS, D).T).astype(NPBF16)

    cosT = fc.T                      # [64, S]
    sinT = fs.T
    cos_dup = np.vstack([cosT, cosT])
    sin_sgn = np.vstack([-sinT, sinT])
    trig = np.ascontiguousarray(np.hstack([cos_dup, sin_sgn])).astype(NPBF16)

    em = np.exp(mask).T              # [k, q]; exp(-inf)=0, exp(0)=1
    emaskT = np.ascontiguousarray(em).astype(NPBF16)
    cls = []
    em_blocks = []
    em_index = {}
    for qw in range(NQW):
        row = []
        for c in range(NKC):
            t = emaskT[c * 128:(c + 1) * 128, qw * W:(qw + 1) * W]
            if not t.any():
                row.append((SKIP, 0, -1))
            elif (t == NPBF16(1.0)).all():
                row.append((FREE, 0, -1))
            else:
                colnz = (np.asarray(t, dtype=np.float32) != 0).any(axis=0)
                off = int(np.argmax(colnz))  # first column with any valid entry
                key = t.tobytes()
                if key not in em_index:
                    em_index[key] = len(em_blocks)
                    em_blocks.append(t)
                row.append((MASKED, off, em_index[key]))
        cls.append(tuple(row))
    cls_key = tuple(cls)
    n_em = len(em_blocks)
    # emu: [128, n_em, 2, W] — each unique block duplicated for both heads
    emu = np.zeros((128, max(n_em, 1), 2, W), dtype=NPBF16)
    for e, blk in enumerate(em_blocks):
        emu[:, e, 0, :] = blk
        emu[:, e, 1, :] = blk

    # deinterleave perm: even dims then odd dims, per head
    ridx = np.concatenate([np.arange(0, HD, 2), np.arange(1, HD, 2)])
    in_maps = []
    for core in range(NCORES):
        heads = [core * HC + h for h in range(HC)]
        qk_rows = np.concatenate([g * HD + ridx for g in heads])
        v_rows = np.concatenate([np.arange(g * HD, (g + 1) * HD) for g in heads])
        m = {
            "xT": xT,
            "wqT": np.ascontiguousarray(wq[qk_rows].T).astype(NPBF16),
            "wkT": np.ascontiguousarray(wk[qk_rows].T).astype(NPBF16),
            "wvT": np.ascontiguousarray(wv[v_rows].T).astype(NPBF16),
            "woT": np.ascontiguousarray(wo[:, v_rows].T).astype(NPBF16),
            "trig": trig,
            "emu": emu,
        }
        in_maps.append(m)
    return in_maps, cls_key, n_em


def kernel(x, start_pos, freqs_cos, freqs_sin, mask, wq, wk, wv, wo):
    in_maps, cls_key, n_em = _prepare(x, freqs_cos, freqs_sin, mask, wq, wk, wv, wo)
    nc = _PROGRAM_CACHE.get(cls_key)
    if nc is None:
        nc = _build(cls_key, n_em)
        _PROGRAM_CACHE[cls_key] = nc
    res = run_bass_kernel_spmd(
        nc, in_maps, list(range(NCORES)),
        trace=bool(os.environ.get("KERNEL_TRACE")),
        tmpdir=os.environ.get("KERNEL_TRACE_DIR") or None)
    LAST_RUN[0] = res
    out = np.zeros([BS, D], np.float32)
    for r in res.results:
        out += np.asarray(r["out"], dtype=np.float32)
    return out.reshape(B, S, D)
